# revision 6
# baseline (speedup 1.0000x reference)
"""Multi-head self-attention (1x1-conv projections, N=4096 spatial tokens,
C=256 channels, Cq=32) on 8 TRN2 NeuronCores, data-parallel over batch.

Per core (one batch element, x as [C, N]):
  q = wq @ x + bq          [Cq, N]
  k = wk @ x + bk          [Cq, N]
  v = wv @ x               [C, N]   (bv folded into the epilogue)
  S = q^T k                [N, N]
  P = softmax(S, axis=-1)
  out = gamma * (v @ P^T + bv) + x

Layout strategy: compute S^T tiles (keys j on partitions, queries i on the
free dim) so softmax's exp output E^T feeds the PV matmul as the stationary
operand with rhs = [v^T | ones]; the ones column accumulates the softmax
denominator for free (no P transposes, no separate reduction). exp skips
max-subtraction: S ~ N(0, 32), |S| < ~40 stays far inside fp32 exp range.

dtypes: fp32r (tf32-like, full PE speed at moving-dim>=256) for the
q/k/energy path where exp amplifies absolute error; bf16 for the P*V path
where softmax normalization cancels it.
"""

import numpy as np

import concourse.bass as bass
import concourse.mybir as mybir
import concourse.tile as tile
from concourse.bass_utils import run_bass_kernel_spmd
from concourse.masks import make_identity
from concourse.tile import ScopedClock

F32 = mybir.dt.float32
F32R = mybir.dt.float32r
BF16 = mybir.dt.bfloat16

B, C, CQ = 8, 256, 32
H = W = 64
N = H * W            # 4096 tokens
NCORES = 8
CT = C // 128        # 2 channel tiles
IB = 512             # queries per i-block
N_IB = N // IB       # 8
JT = N // 128        # 32 key tiles
JGRP = 4             # key tiles per exp group (one PSUM S tile = 4 banks)
N_JG = JT // JGRP    # 8


class PatchedTileContext(tile.TileContext):
    """This walrus build supports only ONE sync-wait command per
    instruction. Peel extra waits into standalone single-wait NOPs on the
    same engine queue, emitted immediately before the instruction (a serial
    conjunction of waits - semantically identical). Same treatment for the
    kernel-tail drain, whose global-clock waits otherwise all land on one
    Drain instruction."""

    MAX_WAITS_PER_INST = 1

    def _add_instruction(self, inst):
        si = inst.sync_info
        waits = list(si.on_wait) if si is not None and si.on_wait else []
        if len(waits) > self.MAX_WAITS_PER_INST and inst.engine is not None:
            keep = waits[-self.MAX_WAITS_PER_INST:]
            peel = waits[: -self.MAX_WAITS_PER_INST]
            for w in peel:
                nop = mybir.InstNoOp(
                    name=self.nc.get_next_instruction_name(),
                    ins=[],
                    outs=[],
                    sync_info=mybir.SyncInfo(on_wait=[w], on_update=[]),
                )
                nop.engine = inst.engine
                super()._add_instruction(nop)
            inst.sync_info = mybir.SyncInfo(
                on_wait=keep,
                on_update=list(si.on_update) if si.on_update else [],
            )
        super()._add_instruction(inst)

    def _drain_and_barrier(self, tick_clock, wait_clock):
        nc = self.nc
        carrier = nc.sync.nop()
        wait_clock.add_sem_waits(
            carrier.ins, ScopedClock({None: tick_clock.global_clock})
        )
        si = carrier.ins.sync_info
        waits = list(si.on_wait) if si is not None and si.on_wait else []
        carrier.ins.sync_info = None
        for w in waits:
            h = bass.SemaphoreHandle(name=w.ant_name or f"sem{w.id}", num=w.id)
            if w.wait_mode == "sem-ge-imm":
                nc.sync.wait_ge(h, w.wait_value)
            else:
                op = {
                    "sem-eq-imm": "eq",
                    "sem-le-imm": "le",
                    "sem-lt-imm": "lt",
                    "sem-gt-imm": "gt",
                }[w.wait_mode]
                nc.sync.wait_op(h, w.wait_value, op)
        nc.sync.drain()
        nc.all_engine_barrier()
        assert self.sems is not None
        popped = nc._tile_sem_poison_stack.pop()
        assert popped is self._sem_poison
        nc.clear_and_free_semaphores(list(self.sems.allocated().values()))
        nc.all_engine_barrier()


def _attention_body(nc, tc, ctx):
    x_e = nc.dram_tensor("x", [C, N], F32, kind="ExternalInput")
    wqt4_e = nc.dram_tensor("wqt4", [C, 128], F32, kind="ExternalInput")
    wkt4_e = nc.dram_tensor("wkt4", [C, 128], F32, kind="ExternalInput")
    wvt_e = nc.dram_tensor("wvt", [C, C], F32, kind="ExternalInput")
    bq4_e = nc.dram_tensor("bq4", [128, 1], F32, kind="ExternalInput")
    bk4_e = nc.dram_tensor("bk4", [128, 1], F32, kind="ExternalInput")
    bv_e = nc.dram_tensor("bv2", [128, CT], F32, kind="ExternalInput")
    gamma_e = nc.dram_tensor("gamma128", [128, 1], F32, kind="ExternalInput")
    out_e = nc.dram_tensor("out", [C, N], F32, kind="ExternalOutput")

    x_v = x_e.rearrange("(t p) n -> p t n", p=128)      # [128, CT, N]
    out_v = out_e.rearrange("(t p) n -> p t n", p=128)  # [128, CT, N]
    wqt_v = wqt4_e.rearrange("(t p) m -> p t m", p=128)
    wkt_v = wkt4_e.rearrange("(t p) m -> p t m", p=128)
    wvt_v = wvt_e.rearrange("(t p) m -> p t m", p=128)

    const = ctx.enter_context(tc.tile_pool(name="const", bufs=1))
    sb = ctx.enter_context(tc.tile_pool(name="sb", bufs=1))
    eps = ctx.enter_context(tc.tile_pool(name="eps", bufs=4))
    outp = ctx.enter_context(tc.tile_pool(name="outp", bufs=4))

    # ---- constants / weights ----
    bq4 = const.tile([128, 1], F32)
    bk4 = const.tile([128, 1], F32)
    bv2 = const.tile([128, CT], F32)
    gamma = const.tile([128, 1], F32)
    nc.sync.dma_start(out=bq4, in_=bq4_e[:, :])
    nc.sync.dma_start(out=bk4, in_=bk4_e[:, :])
    nc.sync.dma_start(out=bv2, in_=bv_e[:, :])
    nc.sync.dma_start(out=gamma, in_=gamma_e[:, :])

    wq_f = const.tile([128, CT, 128], F32)
    wk_f = const.tile([128, CT, 128], F32)
    wv_f = const.tile([128, CT, C], F32)
    nc.sync.dma_start(out=wq_f, in_=wqt_v)
    nc.sync.dma_start(out=wk_f, in_=wkt_v)
    nc.sync.dma_start(out=wv_f, in_=wvt_v)
    wq_r = const.tile([128, CT, 128], F32R)
    wk_r = const.tile([128, CT, 128], F32R)
    wv_r = const.tile([128, CT, C], F32R)
    nc.vector.tensor_copy(out=wq_r, in_=wq_f)
    nc.vector.tensor_copy(out=wk_r, in_=wk_f)
    nc.vector.tensor_copy(out=wv_r, in_=wv_f)

    ident = const.tile([128, 128], BF16)
    make_identity(nc, ident)

    # ---- x load + fp32r round ----
    x_sb = sb.tile([128, CT, N], F32)
    nc.sync.dma_start(out=x_sb, in_=x_v)
    xf_r = sb.tile([128, CT, N], F32R)
    nc.vector.tensor_copy(out=xf_r, in_=x_sb)

    # ---- projections ----
    qT = sb.tile([128, N], F32R)   # q^T replicated on 4 partition groups
    kT = sb.tile([128, N], F32R)
    v1T = sb.tile([128, JT, C + 1], BF16)  # [j-part, j-tile, c | ones]

    with tc.tile_pool(name="psA", bufs=4, space="PSUM") as psA:
        for ch in range(8):
            sl = bass.ts(ch, 512)
            pq = psA.tile([128, 512], F32, tag="pj")
            nc.tensor.matmul(pq, wq_r[:, 0, :], xf_r[:, 0, sl], start=True, stop=False)
            nc.tensor.matmul(pq, wq_r[:, 1, :], xf_r[:, 1, sl], start=False, stop=True)
            nc.vector.tensor_scalar(
                out=qT[:, sl], in0=pq, scalar1=bq4, scalar2=None,
                op0=mybir.AluOpType.add,
            )
            pk = psA.tile([128, 512], F32, tag="pj")
            nc.tensor.matmul(pk, wk_r[:, 0, :], xf_r[:, 0, sl], start=True, stop=False)
            nc.tensor.matmul(pk, wk_r[:, 1, :], xf_r[:, 1, sl], start=False, stop=True)
            nc.vector.tensor_scalar(
                out=kT[:, sl], in0=pk, scalar1=bk4, scalar2=None,
                op0=mybir.AluOpType.add,
            )
        for nt in range(JT):
            pv = psA.tile([128, C], F32, tag="pj")
            nc.tensor.matmul(
                pv, xf_r[:, 0, bass.ts(nt, 128)], wv_r[:, 0, :], start=True, stop=False
            )
            nc.tensor.matmul(
                pv, xf_r[:, 1, bass.ts(nt, 128)], wv_r[:, 1, :], start=False, stop=True
            )
            nc.vector.tensor_copy(out=v1T[:, nt, 0:C], in_=pv)
        nc.vector.memset(v1T[:, :, C : C + 1], 1.0)

    # xb = x + gamma*bv  (residual with bv folded in; written in place)
    gbv = const.tile([128, CT], F32)
    nc.vector.tensor_scalar(
        out=gbv, in0=bv2, scalar1=gamma, scalar2=None, op0=mybir.AluOpType.mult
    )
    for t in range(CT):
        nc.vector.tensor_scalar(
            out=x_sb[:, t, :], in0=x_sb[:, t, :], scalar1=gbv[:, t : t + 1],
            scalar2=None, op0=mybir.AluOpType.add,
        )

    # ---- attention ----
    E = sb.tile([128, JT, IB], BF16)  # exp(S^T) for one i-block

    with (
        tc.tile_pool(name="psS", bufs=1, space="PSUM") as psS,
        tc.tile_pool(name="psO", bufs=4, space="PSUM") as psO,
    ):
        for ib in range(N_IB):
            isl = bass.ds(ib * IB, IB)
            po = [
                psO.tile([128, C + 1], F32, tag="acc", name=f"po_{ib}_{i_s}")
                for i_s in range(4)
            ]
            for jg in range(N_JG):
                ps = psS.tile([128, JGRP, IB], F32, tag="S")
                for g in range(JGRP):
                    jt = jg * JGRP + g
                    gp = bass.ds(32 * g, 32)
                    nc.tensor.matmul(
                        ps[:, g, :],
                        kT[gp, bass.ts(jt, 128)],
                        qT[gp, isl],
                        start=True, stop=True,
                        tile_position=(32 * g, 0),
                    )
                nc.scalar.activation(
                    out=E[:, jg * JGRP : (jg + 1) * JGRP, :],
                    in_=ps[:, :, :],
                    func=mybir.ActivationFunctionType.Exp,
                )
                for g in range(JGRP):
                    jt = jg * JGRP + g
                    for i_s in range(4):
                        nc.tensor.matmul(
                            po[i_s],
                            E[:, jt, bass.ts(i_s, 128)],
                            v1T[:, jt, :],
                            start=(jt == 0), stop=(jt == JT - 1),
                        )
            # epilogue: normalize, transpose to [c, n], residual, store
            for i_s in range(4):
                rd = eps.tile([128, 1], F32, tag="rd")
                nc.vector.reciprocal(out=rd, in_=po[i_s][:, C : C + 1])
                nc.vector.tensor_mul(out=rd, in0=rd, in1=gamma)
                pvn = eps.tile([128, C], BF16, tag="pvn")
                nc.vector.tensor_scalar(
                    out=pvn, in0=po[i_s][:, 0:C], scalar1=rd, scalar2=None,
                    op0=mybir.AluOpType.mult,
                )
                pt = psO.tile([128, C], BF16, tag="acc")
                nc.tensor.transpose(pt[:, 0:128], pvn[:, 0:128], ident)
                nc.tensor.transpose(pt[:, 128:256], pvn[:, 128:256], ident)
                for t in range(CT):
                    ot = outp.tile([128, 128], F32, tag="ot")
                    nc.vector.tensor_add(
                        out=ot,
                        in0=pt[:, bass.ts(t, 128)],
                        in1=x_sb[:, t, bass.ds(ib * IB + i_s * 128, 128)],
                    )
                    nc.sync.dma_start(
                        out=out_v[:, t, bass.ds(ib * IB + i_s * 128, 128)], in_=ot
                    )


_CACHE = {}


def _build():
    if "nc" not in _CACHE:
        nc = bass.Bass()
        from contextlib import ExitStack
        with PatchedTileContext(nc) as tc, ExitStack() as ctx:
            _attention_body(nc, tc, ctx)
        _CACHE["nc"] = nc
    return _CACHE["nc"]


def _prep_in_maps(x, wq, bq, wk, bk, wv, bv, gamma):
    asc = np.ascontiguousarray
    wqt4 = asc(np.tile(wq, (4, 1)).T.astype(np.float32))    # [C, 128]
    wkt4 = asc(np.tile(wk, (4, 1)).T.astype(np.float32))    # [C, 128]
    wvt = asc(wv.T.astype(np.float32))                      # [C, C]
    bq4 = asc(np.tile(bq, 4)[:, None].astype(np.float32))   # [128, 1]
    bk4 = asc(np.tile(bk, 4)[:, None].astype(np.float32))
    bv2 = asc(bv.reshape(CT, 128).T.astype(np.float32))     # [128, CT]
    g128 = np.full((128, 1), np.float32(gamma[0]), dtype=np.float32)
    maps = []
    for b in range(B):
        maps.append({
            "x": asc(x[b].reshape(C, N).astype(np.float32)),
            "wqt4": wqt4, "wkt4": wkt4, "wvt": wvt,
            "bq4": bq4, "bk4": bk4, "bv2": bv2, "gamma128": g128,
        })
    return maps


def _run(inputs, trace=False):
    nc = _build()
    in_maps = _prep_in_maps(**{k: np.asarray(v) for k, v in inputs.items()})
    res = run_bass_kernel_spmd(nc, in_maps, list(range(NCORES)), trace=trace)
    out = np.stack([res.results[b]["out"].reshape(C, H, W) for b in range(B)])
    return out.astype(np.float32), res


def kernel(**inputs):
    out, _ = _run(inputs, trace=False)
    return out


# revision 7
# speedup vs baseline: 1.4760x; 1.4760x over previous
"""Multi-head self-attention (1x1-conv projections, N=4096 spatial tokens,
C=256 channels, Cq=32) on 8 TRN2 NeuronCores, data-parallel over batch.

Per core (one batch element, x as [C, N]):
  q = wq @ x + bq          [Cq, N]
  k = wk @ x + bk          [Cq, N]
  v = wv @ x               [C, N]   (bv folded into the epilogue)
  S = q^T k                [N, N]
  P = softmax(S, axis=-1)
  out = gamma * (v @ P^T + bv) + x

Layout strategy: compute S^T tiles (keys j on partitions, queries i on the
free dim) so softmax's exp output E^T feeds the PV matmul as the stationary
operand with rhs = [v^T | ones]; the ones column accumulates the softmax
denominator for free (no P transposes, no separate reduction). exp skips
max-subtraction: S ~ N(0, 32), |S| < ~40 stays far inside fp32 exp range.

dtypes: fp32r (tf32-like, full PE speed at moving-dim>=256) for the
q/k/energy path where exp amplifies absolute error; bf16 for the P*V path
where softmax normalization cancels it.
"""

import numpy as np

import concourse.bass as bass
import concourse.mybir as mybir
import concourse.tile as tile
from concourse.bass_utils import run_bass_kernel_spmd
from concourse.masks import make_identity
from concourse.tile import ScopedClock

F32 = mybir.dt.float32
F32R = mybir.dt.float32r
BF16 = mybir.dt.bfloat16

B, C, CQ = 8, 256, 32
H = W = 64
N = H * W            # 4096 tokens
NCORES = 8
CT = C // 128        # 2 channel tiles
IB = 512             # queries per i-block
N_IB = N // IB       # 8
JT = N // 128        # 32 key tiles
JGRP = 4             # key tiles per exp group (one PSUM S tile = 4 banks)
N_JG = JT // JGRP    # 8


class PatchedTileContext(tile.TileContext):
    """This walrus build supports only ONE sync-wait command per
    instruction. Peel extra waits into standalone single-wait NOPs on the
    same engine queue, emitted immediately before the instruction (a serial
    conjunction of waits - semantically identical). Same treatment for the
    kernel-tail drain, whose global-clock waits otherwise all land on one
    Drain instruction."""

    MAX_WAITS_PER_INST = 1

    def _add_instruction(self, inst):
        si = inst.sync_info
        waits = list(si.on_wait) if si is not None and si.on_wait else []
        if len(waits) > self.MAX_WAITS_PER_INST and inst.engine is not None:
            keep = waits[-self.MAX_WAITS_PER_INST:]
            peel = waits[: -self.MAX_WAITS_PER_INST]
            for w in peel:
                nop = mybir.InstNoOp(
                    name=self.nc.get_next_instruction_name(),
                    ins=[],
                    outs=[],
                    sync_info=mybir.SyncInfo(on_wait=[w], on_update=[]),
                )
                nop.engine = inst.engine
                super()._add_instruction(nop)
            inst.sync_info = mybir.SyncInfo(
                on_wait=keep,
                on_update=list(si.on_update) if si.on_update else [],
            )
        super()._add_instruction(inst)

    def _drain_and_barrier(self, tick_clock, wait_clock):
        nc = self.nc
        carrier = nc.sync.nop()
        wait_clock.add_sem_waits(
            carrier.ins, ScopedClock({None: tick_clock.global_clock})
        )
        si = carrier.ins.sync_info
        waits = list(si.on_wait) if si is not None and si.on_wait else []
        carrier.ins.sync_info = None
        for w in waits:
            h = bass.SemaphoreHandle(name=w.ant_name or f"sem{w.id}", num=w.id)
            if w.wait_mode == "sem-ge-imm":
                nc.sync.wait_ge(h, w.wait_value)
            else:
                op = {
                    "sem-eq-imm": "eq",
                    "sem-le-imm": "le",
                    "sem-lt-imm": "lt",
                    "sem-gt-imm": "gt",
                }[w.wait_mode]
                nc.sync.wait_op(h, w.wait_value, op)
        nc.sync.drain()
        nc.all_engine_barrier()
        assert self.sems is not None
        popped = nc._tile_sem_poison_stack.pop()
        assert popped is self._sem_poison
        nc.clear_and_free_semaphores(list(self.sems.allocated().values()))
        nc.all_engine_barrier()


def _attention_body(nc, tc, ctx):
    x_e = nc.dram_tensor("x", [C, N], F32, kind="ExternalInput")
    wqt4_e = nc.dram_tensor("wqt4", [C, 128], F32, kind="ExternalInput")
    wkt4_e = nc.dram_tensor("wkt4", [C, 128], F32, kind="ExternalInput")
    wvt_e = nc.dram_tensor("wvt", [C, C], F32, kind="ExternalInput")
    bq4_e = nc.dram_tensor("bq4", [128, 1], F32, kind="ExternalInput")
    bk4_e = nc.dram_tensor("bk4", [128, 1], F32, kind="ExternalInput")
    bv_e = nc.dram_tensor("bv2", [128, CT], F32, kind="ExternalInput")
    gamma_e = nc.dram_tensor("gamma128", [128, 1], F32, kind="ExternalInput")
    out_e = nc.dram_tensor("out", [C, N], F32, kind="ExternalOutput")

    x_v = x_e.rearrange("(t p) n -> p t n", p=128)      # [128, CT, N]
    out_v = out_e.rearrange("(t p) n -> p t n", p=128)  # [128, CT, N]
    wqt_v = wqt4_e.rearrange("(t p) m -> p t m", p=128)
    wkt_v = wkt4_e.rearrange("(t p) m -> p t m", p=128)
    wvt_v = wvt_e.rearrange("(t p) m -> p t m", p=128)

    const = ctx.enter_context(tc.tile_pool(name="const", bufs=1))
    sb = ctx.enter_context(tc.tile_pool(name="sb", bufs=1))
    eps = ctx.enter_context(tc.tile_pool(name="eps", bufs=4))
    outp = ctx.enter_context(tc.tile_pool(name="outp", bufs=4))

    # ---- constants / weights ----
    bq4 = const.tile([128, 1], F32)
    bk4 = const.tile([128, 1], F32)
    bv2 = const.tile([128, CT], F32)
    gamma = const.tile([128, 1], F32)
    nc.sync.dma_start(out=bq4, in_=bq4_e[:, :])
    nc.sync.dma_start(out=bk4, in_=bk4_e[:, :])
    nc.sync.dma_start(out=bv2, in_=bv_e[:, :])
    nc.sync.dma_start(out=gamma, in_=gamma_e[:, :])

    wq_f = const.tile([128, CT, 128], F32)
    wk_f = const.tile([128, CT, 128], F32)
    wv_f = const.tile([128, CT, C], F32)
    nc.sync.dma_start(out=wq_f, in_=wqt_v)
    nc.sync.dma_start(out=wk_f, in_=wkt_v)
    nc.sync.dma_start(out=wv_f, in_=wvt_v)
    wq_r = const.tile([128, CT, 128], F32R)
    wk_r = const.tile([128, CT, 128], F32R)
    wv_r = const.tile([128, CT, C], F32R)
    nc.vector.tensor_copy(out=wq_r, in_=wq_f)
    nc.vector.tensor_copy(out=wk_r, in_=wk_f)
    nc.vector.tensor_copy(out=wv_r, in_=wv_f)

    ident = const.tile([128, 128], BF16)
    make_identity(nc, ident)

    # ---- x load + fp32r round ----
    x_sb = sb.tile([128, CT, N], F32)
    nc.sync.dma_start(out=x_sb, in_=x_v)
    xf_r = sb.tile([128, CT, N], F32R)
    nc.vector.tensor_copy(out=xf_r, in_=x_sb)

    # ---- projections ----
    qT = sb.tile([128, N], F32R)   # q^T replicated on 4 partition groups
    kT = sb.tile([128, N], F32R)
    v1T = sb.tile([128, JT, C + 1], BF16)  # [j-part, j-tile, c | ones]

    with tc.tile_pool(name="psA", bufs=4, space="PSUM") as psA:
        for ch in range(8):
            sl = bass.ts(ch, 512)
            pq = psA.tile([128, 512], F32, tag="pj")
            nc.tensor.matmul(pq, wq_r[:, 0, :], xf_r[:, 0, sl], start=True, stop=False)
            nc.tensor.matmul(pq, wq_r[:, 1, :], xf_r[:, 1, sl], start=False, stop=True)
            nc.vector.tensor_scalar(
                out=qT[:, sl], in0=pq, scalar1=bq4, scalar2=None,
                op0=mybir.AluOpType.add,
            )
            pk = psA.tile([128, 512], F32, tag="pj")
            nc.tensor.matmul(pk, wk_r[:, 0, :], xf_r[:, 0, sl], start=True, stop=False)
            nc.tensor.matmul(pk, wk_r[:, 1, :], xf_r[:, 1, sl], start=False, stop=True)
            nc.vector.tensor_scalar(
                out=kT[:, sl], in0=pk, scalar1=bk4, scalar2=None,
                op0=mybir.AluOpType.add,
            )
        for nt in range(JT):
            pv = psA.tile([128, C], F32, tag="pj")
            nc.tensor.matmul(
                pv, xf_r[:, 0, bass.ts(nt, 128)], wv_r[:, 0, :], start=True, stop=False
            )
            nc.tensor.matmul(
                pv, xf_r[:, 1, bass.ts(nt, 128)], wv_r[:, 1, :], start=False, stop=True
            )
            nc.vector.tensor_copy(out=v1T[:, nt, 0:C], in_=pv)
        nc.vector.memset(v1T[:, :, C : C + 1], 1.0)

    # xb = x + gamma*bv  (residual with bv folded in; written in place)
    gbv = const.tile([128, CT], F32)
    nc.vector.tensor_scalar(
        out=gbv, in0=bv2, scalar1=gamma, scalar2=None, op0=mybir.AluOpType.mult
    )
    for t in range(CT):
        nc.vector.tensor_scalar(
            out=x_sb[:, t, :], in0=x_sb[:, t, :], scalar1=gbv[:, t : t + 1],
            scalar2=None, op0=mybir.AluOpType.add,
        )

    # ---- attention ----
    E = sb.tile([128, JT, IB], BF16)  # exp(S^T) for one i-block

    def emit_energy(ib, jg):
        # S^T for 4 key-tiles (row-packed K=32 matmuls) + one wide exp
        isl = bass.ds(ib * IB, IB)
        ps = psS.tile([128, JGRP, IB], F32, tag="S", name=f"S_{ib}_{jg}")
        for g in range(JGRP):
            jt = jg * JGRP + g
            gp = bass.ds(32 * g, 32)
            nc.tensor.matmul(
                ps[:, g, :],
                kT[gp, bass.ts(jt, 128)],
                qT[gp, isl],
                start=True, stop=True,
                tile_position=(32 * g, 0),
            )
        nc.scalar.activation(
            out=E[:, jg * JGRP : (jg + 1) * JGRP, :],
            in_=ps[:, :, :],
            func=mybir.ActivationFunctionType.Exp,
        )

    with (
        tc.tile_pool(name="psS", bufs=1, space="PSUM") as psS,
        tc.tile_pool(name="psO", bufs=4, space="PSUM") as psO,
    ):
        emit_energy(0, 0)
        for ib in range(N_IB):
            po = [
                psO.tile([128, C + 1], F32, tag="acc", name=f"po_{ib}_{i_s}")
                for i_s in range(4)
            ]
            for jg in range(N_JG):
                # software pipeline: queue the NEXT group's energy+exp ahead
                # of this group's PV matmuls so ACT overlaps the PE stream
                if jg + 1 < N_JG:
                    emit_energy(ib, jg + 1)
                elif ib + 1 < N_IB:
                    emit_energy(ib + 1, 0)
                for g in range(JGRP):
                    jt = jg * JGRP + g
                    for i_s in range(4):
                        nc.tensor.matmul(
                            po[i_s],
                            E[:, jt, bass.ts(i_s, 128)],
                            v1T[:, jt, :],
                            start=(jt == 0), stop=(jt == JT - 1),
                        )
            # epilogue: normalize, transpose to [c, n], residual, store
            for i_s in range(4):
                rd = eps.tile([128, 1], F32, tag="rd")
                nc.vector.reciprocal(out=rd, in_=po[i_s][:, C : C + 1])
                nc.vector.tensor_mul(out=rd, in0=rd, in1=gamma)
                pvn = eps.tile([128, C], BF16, tag="pvn")
                nc.vector.tensor_scalar(
                    out=pvn, in0=po[i_s][:, 0:C], scalar1=rd, scalar2=None,
                    op0=mybir.AluOpType.mult,
                )
                pt = psO.tile([128, C], BF16, tag="acc")
                nc.tensor.transpose(pt[:, 0:128], pvn[:, 0:128], ident)
                nc.tensor.transpose(pt[:, 128:256], pvn[:, 128:256], ident)
                for t in range(CT):
                    ot = outp.tile([128, 128], F32, tag="ot")
                    nc.vector.tensor_add(
                        out=ot,
                        in0=pt[:, bass.ts(t, 128)],
                        in1=x_sb[:, t, bass.ds(ib * IB + i_s * 128, 128)],
                    )
                    nc.sync.dma_start(
                        out=out_v[:, t, bass.ds(ib * IB + i_s * 128, 128)], in_=ot
                    )


_CACHE = {}


def _build():
    if "nc" not in _CACHE:
        nc = bass.Bass()
        from contextlib import ExitStack
        with PatchedTileContext(nc) as tc, ExitStack() as ctx:
            _attention_body(nc, tc, ctx)
        _CACHE["nc"] = nc
    return _CACHE["nc"]


def _prep_in_maps(x, wq, bq, wk, bk, wv, bv, gamma):
    asc = np.ascontiguousarray
    wqt4 = asc(np.tile(wq, (4, 1)).T.astype(np.float32))    # [C, 128]
    wkt4 = asc(np.tile(wk, (4, 1)).T.astype(np.float32))    # [C, 128]
    wvt = asc(wv.T.astype(np.float32))                      # [C, C]
    bq4 = asc(np.tile(bq, 4)[:, None].astype(np.float32))   # [128, 1]
    bk4 = asc(np.tile(bk, 4)[:, None].astype(np.float32))
    bv2 = asc(bv.reshape(CT, 128).T.astype(np.float32))     # [128, CT]
    g128 = np.full((128, 1), np.float32(gamma[0]), dtype=np.float32)
    maps = []
    for b in range(B):
        maps.append({
            "x": asc(x[b].reshape(C, N).astype(np.float32)),
            "wqt4": wqt4, "wkt4": wkt4, "wvt": wvt,
            "bq4": bq4, "bk4": bk4, "bv2": bv2, "gamma128": g128,
        })
    return maps


def _run(inputs, trace=False):
    nc = _build()
    in_maps = _prep_in_maps(**{k: np.asarray(v) for k, v in inputs.items()})
    res = run_bass_kernel_spmd(nc, in_maps, list(range(NCORES)), trace=trace)
    out = np.stack([res.results[b]["out"].reshape(C, H, W) for b in range(B)])
    return out.astype(np.float32), res


def kernel(**inputs):
    out, _ = _run(inputs, trace=False)
    return out
